# revision 7
# baseline (speedup 1.0000x reference)
"""TRN2 Bass kernel for GCNConv-diag: out = A @ (input * diag(W)).

Strategy (8 NeuronCores, SPMD):
  - Shard A row-wise: core i owns rows [i*1024, (i+1)*1024).
  - Replicate the feature matrix `input` (used as the matmul moving
    operand) and W on every core.
  - diag(W) commutes through the matmul: A @ (input*W) == (A @ input)*W,
    so W is applied to the small per-core output during the PSUM drain
    instead of to the 16x larger feature matrix.
  - Matmuls run in float32r (fp32 truncated to 11 mantissa bits): the PE
    processes it at bf16 rate (1 cycle/row for N>=256) with exact fp32
    accumulation in PSUM. Inputs are pre-rounded on the host (required
    by the BIR verifier); this is the only precision loss (~3.6e-4
    Frobenius rel-err vs the fp32 reference).
  - Host pre-arranges A-shards k-major per output tile so every DMA is
    large and fully coalesced (contiguous 16-32KB runs per partition).

Per-core work: out[1024,512] = A_shard[1024,8192] @ x[8192,512], i.e.
512 matmuls of [128k,128m]^T @ [128k,512n] accumulated 64-deep in PSUM.
Roofline: ~52.6MB HBM traffic/core @ ~358GB/s ~= 147us (memory-bound
ridge; PE time ~110-130us hides under it).
"""

import numpy as np

import concourse.bass as bass
import concourse.tile as tile
from concourse import bacc, mybir
from concourse.bass_utils import run_bass_kernel_spmd

N = 8192  # graph nodes (A is [N, N])
D = 512  # feature dim
NCORES = 8
RPC = N // NCORES  # 1024 rows of A / output per core
MT = RPC // 128  # 8 output m-tiles per core
KT = N // 128  # 64 contraction k-tiles
XCH = 8  # x is DMA'd in 8 chunks of 8 k-tiles
KPX = KT // XCH  # k-tiles per x chunk
ACH = 2  # each m-tile's A panel is DMA'd in 2 chunks
KPA = KT // ACH  # k-tiles per a chunk

_F32R = mybir.dt.float32r
_F32 = mybir.dt.float32

_compiled = None
_last_in_maps = None


def _build(repeats=1, mm_dt=_F32R):
    nc = bacc.Bacc("TRN2", target_bir_lowering=False, debug=False, num_devices=NCORES)
    # at[m, p, k*128+c] = A_shard[m*128+c, k*128+p]  (k-on-partitions layout)
    at = nc.dram_tensor("at", [MT, 128, KT * 128], mm_dt, kind="ExternalInput").ap()
    # x[p, k*512+d] = input[k*128+p, d]
    x = nc.dram_tensor("x", [128, KT * D], mm_dt, kind="ExternalInput").ap()
    wb = nc.dram_tensor("wb", [128, D], _F32, kind="ExternalInput").ap()
    out = nc.dram_tensor("out", [RPC, D], _F32, kind="ExternalOutput").ap()

    with tile.TileContext(nc) as tc:
        with (
            tc.tile_pool(name="xp", bufs=1) as xp,
            tc.tile_pool(name="apool", bufs=2 * ACH) as apool,
            tc.tile_pool(name="wp", bufs=1) as wp,
            tc.tile_pool(name="op", bufs=4) as op,
            tc.tile_pool(name="ps", bufs=4, space="PSUM") as ps,
        ):
            for _rep in range(repeats):
                w_t = wp.tile([128, D], _F32, tag="w")
                nc.sync.dma_start(out=w_t[:], in_=wb[:, :])

                # x chunks stay resident for the whole kernel
                # (128KB/partition).
                x_tiles = [None] * XCH

                def load_x(c):
                    xt = xp.tile([128, KPX * D], mm_dt, tag=f"x{c}")
                    nc.sync.dma_start(
                        out=xt[:], in_=x[:, c * KPX * D : (c + 1) * KPX * D]
                    )
                    x_tiles[c] = xt

                def load_a(m):
                    ts = []
                    for c in range(ACH):
                        a_t = apool.tile([128, KPA * 128], mm_dt, tag="a")
                        nc.sync.dma_start(
                            out=a_t[:],
                            in_=at[m, :, c * KPA * 128 : (c + 1) * KPA * 128],
                        )
                        ts.append(a_t)
                    return ts

                # Issue order shapes DMA arrival order (HWDGE drains FIFO):
                # x0, A(m0), x1, A(m1), x2..x7, then A(m2..) in the loop.
                load_x(0)
                a_pending = {0: load_a(0)}
                load_x(1)
                a_pending[1] = load_a(1)
                for c in range(2, XCH):
                    load_x(c)

                for m in range(MT):
                    a_tiles = a_pending.pop(m)
                    psum = ps.tile([128, D], _F32)
                    for k in range(KT):
                        lhsT = a_tiles[k // KPA][
                            :, (k % KPA) * 128 : (k % KPA + 1) * 128
                        ]
                        rhs = x_tiles[k // KPX][:, (k % KPX) * D : (k % KPX + 1) * D]
                        nc.tensor.matmul(
                            psum[:], lhsT, rhs, start=(k == 0), stop=(k == KT - 1)
                        )
                    if m + 2 < MT:
                        a_pending[m + 2] = load_a(m + 2)
                    o_t = op.tile([128, D], _F32)
                    nc.vector.tensor_mul(o_t[:], psum[:], w_t[:])
                    nc.sync.dma_start(
                        out=out[m * 128 : (m + 1) * 128, :], in_=o_t[:]
                    )

    nc.compile()
    return nc


def _get_compiled():
    global _compiled
    if _compiled is None:
        _compiled = _build()
    return _compiled


def _round_f32r(a: np.ndarray) -> np.ndarray:
    """Truncate fp32 to the 11-mantissa-bit FP32R grid (in place on a copy)."""
    a = np.ascontiguousarray(a, dtype=np.float32)
    v = a.view(np.uint32)
    v &= np.uint32(0xFFFFF000)
    return a


def kernel(input, A, W):
    input = np.ascontiguousarray(np.asarray(input, dtype=np.float32))
    A = np.ascontiguousarray(np.asarray(A, dtype=np.float32))
    W = np.ascontiguousarray(np.asarray(W, dtype=np.float32))

    nc = _get_compiled()

    # x[p, k*512+d] = input[k*128+p, d], shared by every core
    xr = _round_f32r(
        input.reshape(KT, 128, D).transpose(1, 0, 2).reshape(128, KT * D)
    )
    wb = np.ascontiguousarray(np.broadcast_to(W[None, :], (128, D)))

    in_maps = []
    for i in range(NCORES):
        a_shard = A[i * RPC : (i + 1) * RPC]
        # atm[m, p, k*128+c] = a_shard[m*128+c, k*128+p]
        atm = _round_f32r(
            a_shard.reshape(MT, 128, KT, 128)
            .transpose(0, 3, 2, 1)
            .reshape(MT, 128, KT * 128)
        )
        in_maps.append({"at": atm, "x": xr, "wb": wb})

    global _last_in_maps
    _last_in_maps = in_maps

    res = run_bass_kernel_spmd(nc, in_maps, list(range(NCORES)))
    return np.concatenate(
        [np.asarray(res.results[i]["out"], dtype=np.float32) for i in range(NCORES)],
        axis=0,
    )


# revision 8
# speedup vs baseline: 1.4110x; 1.4110x over previous
"""TRN2 Bass kernel for GCNConv-diag: out = A @ (input * diag(W)).

Strategy (8 NeuronCores, SPMD):
  - Shard A row-wise: core i owns rows [i*1024, (i+1)*1024).
  - Replicate the feature matrix `input` (used as the matmul moving
    operand) and W on every core.
  - diag(W) commutes through the matmul: A @ (input*W) == (A @ input)*W,
    so W is applied to the small per-core output during the PSUM drain
    instead of to the 16x larger feature matrix.
  - Matmuls run in float32r (fp32 truncated to 11 mantissa bits): the PE
    processes it at bf16 rate (1 cycle/row for N>=256) with exact fp32
    accumulation in PSUM. Inputs are pre-rounded on the host (required
    by the BIR verifier); this is the only precision loss (~3.6e-4
    Frobenius rel-err vs the fp32 reference).
  - Host pre-arranges A-shards k-major per output tile so every DMA is
    large and fully coalesced (contiguous 16-32KB runs per partition).

Per-core work: out[1024,512] = A_shard[1024,8192] @ x[8192,512], i.e.
512 matmuls of [128k,128m]^T @ [128k,512n] accumulated 64-deep in PSUM.
Roofline: ~52.6MB HBM traffic/core @ ~358GB/s ~= 147us (memory-bound
ridge; PE time ~110-130us hides under it).
"""

import numpy as np

import concourse.bass as bass
import concourse.tile as tile
from concourse import bacc, mybir
from concourse.bass_utils import run_bass_kernel_spmd

N = 8192  # graph nodes (A is [N, N])
D = 512  # feature dim
NCORES = 8
RPC = N // NCORES  # 1024 rows of A / output per core
MT = RPC // 128  # 8 output m-tiles per core
KT = N // 128  # 64 contraction k-tiles
XCH = 8  # x is DMA'd in 8 chunks of 8 k-tiles
KPX = KT // XCH  # k-tiles per x chunk
ACH = 2  # each m-tile's A panel is DMA'd in 2 chunks
KPA = KT // ACH  # k-tiles per a chunk

_F32R = mybir.dt.float32r
_F32 = mybir.dt.float32

_compiled = None
_last_in_maps = None


def _build(repeats=1, mm_dt=_F32R):
    nc = bacc.Bacc("TRN2", target_bir_lowering=False, debug=False, num_devices=NCORES)
    # at[m, p, k*128+c] = A_shard[m*128+c, k*128+p]  (k-on-partitions layout)
    at = nc.dram_tensor("at", [MT, 128, KT * 128], mm_dt, kind="ExternalInput").ap()
    # x[p, k*512+d] = input[k*128+p, d]
    x = nc.dram_tensor("x", [128, KT * D], mm_dt, kind="ExternalInput").ap()
    wb = nc.dram_tensor("wb", [128, D], _F32, kind="ExternalInput").ap()
    out = nc.dram_tensor("out", [RPC, D], _F32, kind="ExternalOutput").ap()

    with tile.TileContext(nc) as tc:
        with (
            tc.tile_pool(name="xp", bufs=1) as xp,
            tc.tile_pool(name="apool", bufs=2 * ACH) as apool,
            tc.tile_pool(name="wp", bufs=1) as wp,
            tc.tile_pool(name="op", bufs=4) as op,
            tc.tile_pool(name="ps", bufs=4, space="PSUM") as ps,
        ):
            for _rep in range(repeats):
                w_t = wp.tile([128, D], _F32, tag="w")
                nc.sync.dma_start(out=w_t[:], in_=wb[:, :])

                # x chunks stay resident for the whole kernel
                # (128KB/partition).
                x_tiles = [None] * XCH

                def load_x(c):
                    xt = xp.tile([128, KPX * D], mm_dt, tag=f"x{c}")
                    nc.sync.dma_start(
                        out=xt[:], in_=x[:, c * KPX * D : (c + 1) * KPX * D]
                    )
                    x_tiles[c] = xt

                def load_a(m):
                    ts = []
                    for c in range(ACH):
                        a_t = apool.tile([128, KPA * 128], mm_dt, tag="a")
                        nc.sync.dma_start(
                            out=a_t[:],
                            in_=at[m, :, c * KPA * 128 : (c + 1) * KPA * 128],
                        )
                        ts.append(a_t)
                    return ts

                # Issue order shapes DMA arrival order (HWDGE drains FIFO):
                # x0, A(m0), x1, A(m1), x2..x7, then A(m2..) in the loop.
                load_x(0)
                a_pending = {0: load_a(0)}
                load_x(1)
                a_pending[1] = load_a(1)
                for c in range(2, XCH):
                    load_x(c)

                for m in range(MT):
                    a_tiles = a_pending.pop(m)
                    psum = ps.tile([128, D], _F32)
                    for k in range(KT):
                        lhsT = a_tiles[k // KPA][
                            :, (k % KPA) * 128 : (k % KPA + 1) * 128
                        ]
                        rhs = x_tiles[k // KPX][:, (k % KPX) * D : (k % KPX + 1) * D]
                        nc.tensor.matmul(
                            psum[:], lhsT, rhs, start=(k == 0), stop=(k == KT - 1)
                        )
                    if m + 2 < MT:
                        a_pending[m + 2] = load_a(m + 2)
                    o_t = op.tile([128, D], _F32)
                    nc.vector.tensor_mul(o_t[:], psum[:], w_t[:])
                    nc.sync.dma_start(
                        out=out[m * 128 : (m + 1) * 128, :], in_=o_t[:]
                    )

    nc.compile()
    return nc


def _get_compiled():
    global _compiled
    if _compiled is None:
        _compiled = _build(mm_dt=mybir.dt.float16)
    return _compiled


def kernel(input, A, W):
    input = np.ascontiguousarray(np.asarray(input, dtype=np.float32))
    A = np.ascontiguousarray(np.asarray(A, dtype=np.float32))
    W = np.ascontiguousarray(np.asarray(W, dtype=np.float32))

    nc = _get_compiled()

    # x[p, k*512+d] = input[k*128+p, d], shared by every core
    xr = (
        input.reshape(KT, 128, D)
        .transpose(1, 0, 2)
        .reshape(128, KT * D)
        .astype(np.float16)
    )
    wb = np.ascontiguousarray(np.broadcast_to(W[None, :], (128, D)))

    in_maps = []
    for i in range(NCORES):
        a_shard = A[i * RPC : (i + 1) * RPC]
        # atm[m, p, k*128+c] = a_shard[m*128+c, k*128+p]
        atm = (
            a_shard.reshape(MT, 128, KT, 128)
            .transpose(0, 3, 2, 1)
            .reshape(MT, 128, KT * 128)
            .astype(np.float16)
        )
        in_maps.append({"at": atm, "x": xr, "wb": wb})

    global _last_in_maps
    _last_in_maps = in_maps

    res = run_bass_kernel_spmd(nc, in_maps, list(range(NCORES)))
    return np.concatenate(
        [np.asarray(res.results[i]["out"], dtype=np.float32) for i in range(NCORES)],
        axis=0,
    )
